# revision 1
# baseline (speedup 1.0000x reference)
"""Enc-Dec MultiHeadAttention Bass/Trainium2 kernel, 8-core SPMD.

Problem: B=4, S=2048, EMB=1024, HEADS=16 (d_head=64).
  q = x_dec @ Wq.T + bq ; k = x_dec @ Wk.T + bk ; v = x_enc @ Wv.T + bv
  out = softmax(q k^T / sqrt(EMB)) v  -> @ Wp.T + bp

Sharding: core c handles batch b = c % 4 and head-group g = c // 4
(8 heads = 512 features per group).  Each core computes the attention
output for its (batch, head-group) and the partial output projection
against Wp[:, g*512:(g+1)*512]; the host sums the two partials per batch
and adds bp (standard tensor-parallel gather).

Device-side layout choices (all matmuls contract over the partition dim):
  - q/k are produced feature-major (qT/kT [feat, seq]) so attention
    energies come out k-major: eT[k, q] = kT_h.T-contraction with qT_h
    (K = d_head = 64, two heads packed in the 128-row array via
    tile_position row tiling).
  - exp runs on ScalarE directly from PSUM ([128, 1024] per op covering
    both heads) with scale=1/32 folded into the activation.
  - v is produced seq-major with a ones-column appended per head
    (v_aug[k, 65]); PV matmul (v_aug stationary, attT moving) yields
    outT_unnorm rows 0..63 plus the softmax denominator in row 64 of the
    same PSUM accumulation.
  - normalization: reciprocal of denominators on VectorE, broadcast
    across 128 partitions via a tiny K=2 ones-matmul, one
    tensor_tensor multiply.
"""

import numpy as np
import ml_dtypes
from contextlib import ExitStack

EMB = 1024
S = 2048
B = 4
NCORES = 8
D = 64
HPC = 8            # heads per core
FG = HPC * D       # 512 features per head-group
NPAIRS = HPC // 2  # head pairs per core (row-tiled in the PE array)
CH = 512           # q-chunk width
NCH = S // CH      # 4
ET = EMB // 128    # 8 contraction tiles for the projections
ST = S // 128      # 16 seq tiles
SCALE = float(EMB) ** -0.5  # 1/32

_cache = {}


def _mha_body(tc, ctx, xdT, xeT, wqT, wkT, wvT, wpT, bq, bk, bv, out):
    import concourse.bass as bass
    from concourse import mybir

    nc = tc.nc
    f32 = mybir.dt.float32
    bf16 = mybir.dt.bfloat16
    Exp = mybir.ActivationFunctionType.Exp
    mult = mybir.AluOpType.mult

    wpool = ctx.enter_context(tc.tile_pool(name="weights", bufs=1))
    vpool = ctx.enter_context(tc.tile_pool(name="vaug", bufs=1))
    xpool = ctx.enter_context(tc.tile_pool(name="xstream", bufs=1))
    qkpool = ctx.enter_context(tc.tile_pool(name="qk", bufs=2))
    attpool = ctx.enter_context(tc.tile_pool(name="att", bufs=2))
    opool = ctx.enter_context(tc.tile_pool(name="outu", bufs=1))
    dnpool = ctx.enter_context(tc.tile_pool(name="dn", bufs=1))
    stpool = ctx.enter_context(tc.tile_pool(name="stage", bufs=3))
    ps_mm = ctx.enter_context(tc.tile_pool(name="psmm", bufs=2, space="PSUM"))
    ps_et = ctx.enter_context(tc.tile_pool(name="pset", bufs=2, space="PSUM"))
    ps_pv = ctx.enter_context(tc.tile_pool(name="pspv", bufs=1, space="PSUM"))

    # --- weights / constants -------------------------------------------------
    # emission order matters for DMA queue scheduling: the v-phase consumes
    # xe/wv e-tile by e-tile, so those DMAs go first, interleaved.
    # PE warmup first — no DMA dependencies, so it runs at t~1us: ~5us of
    # dense matmuls flip the HAM clock gate to 2.4GHz before the real work
    # (otherwise the whole v+proj startup runs at 1.2GHz)
    warm_t = wpool.tile([128, CH], bf16, tag="warm")
    nc.vector.memset(warm_t[:], 0.5)
    for i in range(22):
        pw = ps_mm.tile([128, CH], f32, tag="mm", name="pw")
        nc.tensor.matmul(pw[:], warm_t[:, 0:128], warm_t[:], start=True, stop=True)

    # DMA queue order = emission order: tiny bias tensors first (the v-bias
    # matmul would otherwise stall the in-order PE stream on them), then the
    # v-phase inputs (the full-width v projection starts within a few us),
    # then the pair-0 projection inputs, which land while v computes.
    bq_sb = wpool.tile([128, FG // 128], f32, tag="bq")
    nc.sync.dma_start(bq_sb[:], bq.rearrange("(o p) -> p o", p=128))
    bk_sb = wpool.tile([128, FG // 128], f32, tag="bk")
    nc.sync.dma_start(bk_sb[:], bk.rearrange("(o p) -> p o", p=128))
    bv_f = wpool.tile([1, FG], f32, tag="bvf")
    nc.sync.dma_start(bv_f[:], bv.rearrange("(o f) -> o f", o=1))
    bv_sb = wpool.tile([1, FG], bf16, tag="bv")
    nc.vector.tensor_copy(bv_sb[:], bv_f[:])

    xe_sb = xpool.tile([128, ET, S], bf16, tag="x")
    xeTr = xeT.rearrange("(o p) s -> p o s", p=128)
    wv_sb = wpool.tile([128, ET, FG], bf16, tag="wv")
    wvTr = wvT.rearrange("(o p) f -> p o f", p=128)
    for e in range(ET):
        nc.sync.dma_start(wv_sb[:, e], wvTr[:, e])
        nc.sync.dma_start(xe_sb[:, e], xeTr[:, e])
    wq_sb = wpool.tile([128, ET, FG], bf16, tag="wq")
    wqTr = wqT.rearrange("(o p) f -> p o f", p=128)
    wk_sb = wpool.tile([128, ET, FG], bf16, tag="wk")
    wkTr = wkT.rearrange("(o p) f -> p o f", p=128)
    xd_sb = xpool.tile([128, ET, S], bf16, tag="xd")
    xdTr = xdT.rearrange("(o p) s -> p o s", p=128)
    for e in range(ET):
        nc.sync.dma_start(wq_sb[:, e], wqTr[:, e])
        nc.sync.dma_start(wk_sb[:, e], wkTr[:, e])
        nc.sync.dma_start(xd_sb[:, e], xdTr[:, e])
    wp_sb = wpool.tile([128, FG // 128, EMB], bf16, tag="wp")
    nc.sync.dma_start(wp_sb[:], wpT.rearrange("(o p) f -> p o f", p=128))

    ones_v = wpool.tile([1, 128], bf16, tag="ones_v")
    nc.vector.memset(ones_v[:], 1.0)
    # ones2[0, 0:64] = 1, ones2[1, 64:128] = 1: broadcasts the two heads'
    # per-q reciprocals onto the pair's 128 feature partitions.
    ones2 = wpool.tile([2, 128], f32, tag="ones2")
    nc.vector.memset(ones2[:], 0.0)
    nc.vector.memset(ones2[0:1, 0:64], 1.0)
    # engine ops can't start at partition 1; DMA is partition-free
    nc.sync.dma_start(ones2[1:2, 64:128], ones2[0:1, 0:64])

    # --- V projection buffers (emitted below, after the cascade defs) -------
    v_aug = vpool.tile([128, ST, HPC * 65], bf16, tag="vaug")
    va4 = v_aug.rearrange("p s (h f) -> p s h f", f=65)
    nc.vector.memset(va4[:, :, :, 64:65], 1.0)

    # --- persistent attention-output + denominator buffers ------------------
    # outu[p, pair, s]: partitions = pair-local features (64*j + d)
    outu = opool.tile([128, NPAIRS, S], bf16, tag="outu")
    # one row per (head, chunk) at partition 32*pair + 4*j + ch, so each
    # pair's rows start 32-aligned (engine ops need aligned start partitions)
    dn_sp = dnpool.tile([128, CH], f32, tag="dn")
    rc_sp = dnpool.tile([128, CH], f32, tag="rc")
    # only 8 rows per 32-row block get real denominators; init the rest so
    # the batched reciprocal over all 128 partitions reads defined values
    nc.vector.memset(dn_sp[:], 1.0)

    def normalize_chunk(pair, ch):
        cs = slice(ch * CH, (ch + 1) * CH)
        rc_t = dnpool.tile([2, CH], f32, tag="rct", bufs=2, name="rc_t")
        for j in range(2):
            r = 32 * pair + 4 * j + ch
            nc.sync.dma_start(rc_t[j:j + 1, :], rc_sp[r:r + 1, :])
        pb = ps_mm.tile([128, CH], f32, tag="mm", name="pb")
        nc.tensor.matmul(pb[:], ones2[:], rc_t[:], start=True, stop=True)
        nc.vector.tensor_tensor(outu[:, pair, cs], outu[:, pair, cs], pb[:], mult)

    norm_q = []

    def proj_piece(ppair, which, ch, qT_, kT_):
        """One 512-col chunk of the q or k projection for pair `ppair`."""
        fs_ = slice(ppair * 128, (ppair + 1) * 128)
        cs_ = slice(ch * CH, (ch + 1) * CH)
        w_sb, b_sb, dst = ((wq_sb, bq_sb, qT_) if which == "q"
                          else (wk_sb, bk_sb, kT_))
        pp = ps_mm.tile([128, CH], f32, tag="mm", name="pp")
        for e in range(ET):
            nc.tensor.matmul(
                pp[:], w_sb[:, e, fs_], xd_sb[:, e, cs_],
                start=(e == 0), stop=(e == ET - 1),
            )
        nc.vector.tensor_scalar_add(dst[:, cs_], pp[:], b_sb[:, ppair:ppair + 1])

    # --- projection cascade -------------------------------------------------
    # q is consumed chunk-locally (one piece ahead suffices); k is consumed
    # across the full key range by every chunk, so the next pair's k pieces
    # spread across the current pair's four chunks.
    qk_tiles = {}

    def get_qk(p):
        if p not in qk_tiles:
            q_ = qkpool.tile([128, S], bf16, tag="qT", name=f"qT{p}")
            k_ = qkpool.tile([128, S], bf16, tag="kT", name=f"kT{p}")
            qk_tiles[p] = (q_, k_)
        return qk_tiles[p]

    def piece(p_, w_, c_):
        q_, k_ = get_qk(p_)
        proj_piece(p_, w_, c_, q_, k_)

    # --- V projection (seq-major, with ones column per head) ----------------
    # full-width (all heads, N=512): DMA-paced and PE-efficient; pair-0's
    # projection pieces ride the tail of this stream (their inputs are the
    # last DMAs to land anyway)
    for kt in range(ST):
        pvf = ps_mm.tile([128, FG], f32, tag="mm", name="pvf")
        for e in range(ET):
            nc.tensor.matmul(
                pvf[:], xe_sb[:, e, kt * 128:(kt + 1) * 128], wv_sb[:, e, :],
                start=(e == 0), stop=False,
            )
        nc.tensor.matmul(pvf[:], ones_v[:], bv_sb[:], start=False, stop=True)
        nc.vector.tensor_copy(
            va4[:, kt, :, 0:64], pvf.rearrange("p (h f) -> p h f", f=64))

    piece(0, "q", 0)
    for c in range(NCH):
        piece(0, "k", c)

    for pair in range(NPAIRS):
        qT, kT = get_qk(pair)
        if pair == NPAIRS - 1:
            # queue pairs 0..2 normalization; popped inside pair 3's
            # attention stream so the rc chains resolve off the PE path
            nc.vector.reciprocal(rc_sp[0:96, :], dn_sp[0:96, :])
            norm_q.extend((p_, c_) for p_ in range(NPAIRS - 1)
                          for c_ in range(NCH))

        # --- attention for the pair -----------------------------------------
        for ch in range(NCH):
            cs = slice(ch * CH, (ch + 1) * CH)
            ppv = [ps_pv.tile([65, CH], f32, tag=f"pv{j}", name=f"ppv{j}")
                   for j in range(2)]
            attks = {}

            def pv_step(kt):
                attk = attks.pop(kt)
                for j in range(2):
                    h = 2 * pair + j
                    nc.tensor.matmul(
                        ppv[j][:], v_aug[:, kt, h * 65:(h + 1) * 65],
                        attk[:, j * CH:(j + 1) * CH],
                        start=(kt == 0), stop=(kt == ST - 1),
                    )

            for kt in range(ST):
                ks = slice(kt * 128, (kt + 1) * 128)
                eT = ps_et.tile([128, 2 * CH], f32, tag="eT")
                nc.tensor.matmul(
                    eT[:, 0:CH], kT[0:64, ks], qT[0:64, cs],
                    start=True, stop=True,
                )
                nc.tensor.matmul(
                    eT[:, CH:2 * CH], kT[64:128, ks], qT[64:128, cs],
                    start=True, stop=True,
                )
                attk = attpool.tile([128, 2 * CH], bf16, tag="attT", bufs=4)
                nc.scalar.activation(attk[:], eT[:], Exp, scale=SCALE)
                attks[kt] = attk
                # PV runs 2 k-tiles behind exp: at chunk boundaries the next
                # chunk's QK issues before the previous PVs + copies resolve,
                # so the in-order PE never stalls the ACT exp stream
                if kt >= 2:
                    pv_step(kt - 2)
                # the projection cascade
                if kt == 1 and ch < NCH - 1:
                    piece(pair, "q", ch + 1)
                if kt == 9 and pair < NPAIRS - 1:
                    piece(pair + 1, "k", ch)
                if kt == 5 and ch == NCH - 1 and pair < NPAIRS - 1:
                    piece(pair + 1, "q", 0)
                # ... and the deferred normalizations during the last pair
                # (not in chunk 0: the batched reciprocal needs ~5us of DVE
                # before the first rc chain is ready)
                if norm_q and kt % 2 == 1 and ch > 0:
                    normalize_chunk(*norm_q.pop(0))
            pv_step(ST - 2)
            pv_step(ST - 1)
            for j in range(2):
                nc.vector.tensor_copy(
                    outu[64 * j:64 * (j + 1), pair, cs], ppv[j][0:64, :])
                # denominator row: DVE stays in lane 64 (engines can't cross
                # partitions), then a tiny DMA moves it to its dn_sp row
                dn_st = dnpool.tile([128, CH], f32, tag="dnstage", bufs=2)
                nc.vector.tensor_copy(dn_st[64:65, :], ppv[j][64:65, :])
                r = 32 * pair + 4 * j + ch
                nc.sync.dma_start(dn_sp[r:r + 1, :], dn_st[64:65, :])
            if pair == NPAIRS - 1 and ch < NCH - 1:
                # last pair: reciprocal as each chunk's denominators land;
                # the normalize itself queues behind the pair-0..2 items
                nc.vector.reciprocal(rc_sp[96:128, :], dn_sp[96:128, :])
                norm_q.append((pair, ch))

    # --- tail: outproj for chunks 0..2 first (they don't need the last
    # chunk's normalization), the (3,3) normalize chain resolves meanwhile --
    nc.vector.reciprocal(rc_sp[96:128, :], dn_sp[96:128, :])
    for item in norm_q:  # any leftovers for pairs 0..2 / (3, ch<3)
        normalize_chunk(*item)
    norm_q.clear()

    def outproj(qt):
        # both 512-wide output halves of one q-tile share a 2-bank PSUM
        # tile: 8 matmuls, then ONE 1024-wide copy + DMA
        qs = slice(qt * 128, (qt + 1) * 128)
        po = ps_et.tile([128, 2 * CH], f32, tag="eT", name="po")
        for ot in range(EMB // CH):
            os_ = slice(ot * CH, (ot + 1) * CH)
            for pk in range(NPAIRS):
                nc.tensor.matmul(
                    po[:, os_], outu[:, pk, qs], wp_sb[:, pk, os_],
                    start=(pk == 0), stop=(pk == NPAIRS - 1),
                )
        so = stpool.tile([128, 2 * CH], f32, tag="so")
        if qt % 2 == 0:  # split tail copies across the two idle engines
            nc.scalar.copy(so[:], po[:])
        else:
            nc.vector.tensor_copy(so[:], po[:])
        nc.sync.dma_start(out[qs, :], so[:])

    for qt in range(3 * ST // 4):
        outproj(qt)
    normalize_chunk(NPAIRS - 1, NCH - 1)
    for qt in range(3 * ST // 4, ST):
        outproj(qt)


def build():
    """Build + compile the per-core Bass program (cached)."""
    if "nc" in _cache:
        return _cache["nc"]
    import concourse.tile as tile
    from concourse import bacc, mybir

    f32 = mybir.dt.float32
    bf16 = mybir.dt.bfloat16
    nc = bacc.Bacc("TRN2", target_bir_lowering=False, debug=False,
                   num_devices=NCORES)
    xdT = nc.dram_tensor("xdT", (EMB, S), bf16, kind="ExternalInput").ap()
    xeT = nc.dram_tensor("xeT", (EMB, S), bf16, kind="ExternalInput").ap()
    wqT = nc.dram_tensor("wqT", (EMB, FG), bf16, kind="ExternalInput").ap()
    wkT = nc.dram_tensor("wkT", (EMB, FG), bf16, kind="ExternalInput").ap()
    wvT = nc.dram_tensor("wvT", (EMB, FG), bf16, kind="ExternalInput").ap()
    wpT = nc.dram_tensor("wpT", (FG, EMB), bf16, kind="ExternalInput").ap()
    bq = nc.dram_tensor("bq", (FG,), f32, kind="ExternalInput").ap()
    bk = nc.dram_tensor("bk", (FG,), f32, kind="ExternalInput").ap()
    bv = nc.dram_tensor("bv", (FG,), f32, kind="ExternalInput").ap()
    out = nc.dram_tensor("out", (S, EMB), f32, kind="ExternalOutput").ap()

    with tile.TileContext(nc) as tc:
        with ExitStack() as ctx:
            _mha_body(tc, ctx, xdT, xeT, wqT, wkT, wvT, wpT, bq, bk, bv, out)
    nc.compile()
    _cache["nc"] = nc
    return nc


def make_in_maps(x_enc, x_dec, Wq, bq, Wk, bk, Wv, bv, Wp):
    """Host-side sharding: per-core input dict for core c = (g = c//4, b = c%4)."""
    bf = ml_dtypes.bfloat16
    in_maps = []
    xdTs = [np.ascontiguousarray(x_dec[b].T).astype(bf) for b in range(B)]
    xeTs = [np.ascontiguousarray(x_enc[b].T).astype(bf) for b in range(B)]
    for c in range(NCORES):
        g, b = divmod(c, B)
        gs = slice(g * FG, (g + 1) * FG)
        in_maps.append({
            "xdT": xdTs[b],
            "xeT": xeTs[b],
            "wqT": np.ascontiguousarray(Wq[gs].T).astype(bf),
            "wkT": np.ascontiguousarray(Wk[gs].T).astype(bf),
            "wvT": np.ascontiguousarray(Wv[gs].T).astype(bf),
            "wpT": np.ascontiguousarray(Wp[:, gs].T).astype(bf),
            "bq": np.ascontiguousarray(bq[gs]).astype(np.float32),
            "bk": np.ascontiguousarray(bk[gs]).astype(np.float32),
            "bv": np.ascontiguousarray(bv[gs]).astype(np.float32),
        })
    return in_maps


def kernel(x_enc, x_dec, Wq, bq, Wk, bk, Wv, bv, Wp, bp):
    from concourse.bass_utils import run_bass_kernel_spmd

    x_enc = np.asarray(x_enc, dtype=np.float32)
    x_dec = np.asarray(x_dec, dtype=np.float32)
    nc = build()
    in_maps = make_in_maps(np.asarray(x_enc), np.asarray(x_dec),
                           np.asarray(Wq), np.asarray(bq), np.asarray(Wk),
                           np.asarray(bk), np.asarray(Wv), np.asarray(bv),
                           np.asarray(Wp))
    res = run_bass_kernel_spmd(nc, in_maps, core_ids=list(range(NCORES)))
    out = np.empty((B, S, EMB), dtype=np.float32)
    bp32 = np.asarray(bp, dtype=np.float32)
    for b in range(B):
        out[b] = res.results[b]["out"] + res.results[b + B]["out"] + bp32
    return out

